# revision 28
# baseline (speedup 1.0000x reference)
"""Trainium2 Bass kernel for BlockDiagonalAggregator (moe_routing).

Computes, for each batch row b:
    logit[b,k] = dot(keys[sigma[b,k]], h[b,k,:])   (masked -inf where sigma==64)
    alpha      = softmax_k(logit)
    out[b,:]   = sum_k alpha[b,k] * h[b,k,:]

Distribution: data-parallel over B across 8 NeuronCores (512 rows each),
keys replicated, no collectives (per the data-parallel sharding hint).

Per-core algorithm (v3, diag-E layout):
  - b-block = 128 batch rows; 4 blocks per core; chunk = one slot k of a
    block's 128 rows -> 64 chunks per block, 256 per core.  h is shipped
    bf16 in (block, b-part, k, d) order -- a pure reshape+cast on host --
    so each partition's block-load is one contiguous 64KB run.
  - w gather via one-hot matmul on PE: w = oh_c.T @ keys (bf16), where
    oh[a, b] = (sigma[b, c] == a), built host-side.  sigma==64 (unassigned)
    matches no agent -> w = 0.
  - logit via one fused DVE scalar_tensor_tensor: (h*1)*w with free-axis
    accumulate -> logit column [128,1] fp32.
  - e = exp(logit + pen) on ACT, one [128,1] instr per chunk with the
    pen column as the per-partition bias (pen = -1e9 for unassigned ->
    e = 0, matching the reference's mask+softmax; no max-subtraction
    needed since keys std 0.01 keeps |logit| < ~2).  e stored bf16.
  - GpSimd affine_select expands the e column into diag(e) [128,128] bf16
    (iota p-f == 0 keeps the broadcast e, else fills 0).
  - PE pooling: pool_ps += diag(e_c).T @ h_c accumulated over the block's
    64 chunks in a single PSUM bank -> [128, 512].
  - esum = free-axis reduce of the block's e tile [128,64] (one DVE
    instr; replaces per-chunk esum matmuls), recip on DVE,
    out = pool * recip, DMA out fp32.
  - Software pipeline: pool matmul for chunk g runs LAG chunks behind the
    gather/dot/exp/diag front so the PE never waits on the 4-engine chain.
"""

import numpy as np
import ml_dtypes

# Problem constants (hardcoded: kernel.py must be self-contained)
B, K, D = 4096, 64, 512
N_AGENTS = 64
N_CORES = 8
B_CORE = B // N_CORES            # 512 batch rows per core
BLK = 128                        # batch rows per block (= PE partition dim)
N_BLOCKS = B_CORE // BLK         # 4
NEG_BIG = -1e9
LAG = 8                          # pool-matmul pipeline lag (chunks)
KEY_SCALE = 64.0                 # host pre-scale so fp8 keys stay normal-range

_prog_cache = {}


def _build_program():
    import concourse.bacc as bacc
    import concourse.tile as tile
    import concourse.mybir as mybir

    f32 = mybir.dt.float32
    bf16 = mybir.dt.bfloat16
    fp8 = mybir.dt.float8e4
    AF = mybir.ActivationFunctionType
    ALU = mybir.AluOpType
    AX = mybir.AxisListType
    PM = mybir.MatmulPerfMode

    nc = bacc.Bacc("TRN2", target_bir_lowering=False, debug=False,
                   num_devices=N_CORES)

    h_d = nc.dram_tensor("h", [N_BLOCKS, BLK, K * D], bf16,
                         kind="ExternalInput").ap()
    oh_d = nc.dram_tensor("oh", [N_BLOCKS, N_AGENTS, K * BLK], bf16,
                          kind="ExternalInput").ap()
    pen_d = nc.dram_tensor("pen", [BLK, N_BLOCKS * K], f32,
                           kind="ExternalInput").ap()
    keys_d = nc.dram_tensor("keys", [N_AGENTS, D], bf16,
                            kind="ExternalInput").ap()
    out_d = nc.dram_tensor("out", [B_CORE, D], f32, kind="ExternalOutput").ap()

    hd4 = h_d.rearrange("m p (c d) -> m p c d", d=D)
    ohd4 = oh_d.rearrange("m a (c p) -> m a c p", p=BLK)

    with tile.TileContext(nc) as tc:
        with (
            tc.tile_pool(name="const", bufs=1) as const_pool,
            tc.tile_pool(name="hp", bufs=2) as h_pool,
            tc.tile_pool(name="ohp", bufs=2) as oh_pool,
            tc.tile_pool(name="tmpp", bufs=2) as tmp_pool,
            tc.tile_pool(name="logitp", bufs=2) as logit_pool,
            tc.tile_pool(name="ep", bufs=2) as e_pool,
            tc.tile_pool(name="diagp", bufs=8) as diag_pool,
            tc.tile_pool(name="outp", bufs=2) as out_pool,
            tc.tile_pool(name="redp", bufs=2) as red_pool,
            tc.tile_pool(name="psw", bufs=6, space="PSUM") as psw,
            tc.tile_pool(name="psp", bufs=2, space="PSUM") as psp,
        ):
            keys_t = const_pool.tile([N_AGENTS, D], bf16)
            nc.sync.dma_start(keys_t[:], keys_d[:])
            pen_t = const_pool.tile([BLK, N_BLOCKS * K], f32)
            nc.sync.dma_start(pen_t[:], pen_d[:])

            # per-block state, indexed by block id
            h_ts = [None] * N_BLOCKS
            oh_ts = [None] * N_BLOCKS
            logit_ts = [None] * N_BLOCKS
            e_ts = [None] * N_BLOCKS
            pool_ps_s = [None] * N_BLOCKS
            diag_ts = [None] * (N_BLOCKS * K)

            total = N_BLOCKS * K
            GRP = 4   # batch gathers/pools to amortize PE DR<->normal switches
            for base in range(0, total + LAG, GRP):
              for g in range(base, base + GRP):
                if g < total:
                    m, c = divmod(g, K)
                    if c == 0:
                        h_t = h_pool.tile([BLK, K, D], bf16, tag="h")
                        q = K // 4
                        for i in range(4):
                            nc.sync.dma_start(h_t[:, i * q:(i + 1) * q, :],
                                              hd4[m][:, i * q:(i + 1) * q, :])
                        h_ts[m] = h_t
                        oh_t = oh_pool.tile([N_AGENTS, K, BLK], bf16,
                                            tag="oh")
                        nc.sync.dma_start(oh_t[:], ohd4[m])
                        oh_ts[m] = oh_t
                        logit_ts[m] = logit_pool.tile([BLK, K], f32,
                                                      tag="logit", name="logit_t")
                        e_ts[m] = e_pool.tile([BLK, K], bf16, tag="e",
                                              name="e_t")
                        pool_ps_s[m] = psp.tile([BLK, D], f32, tag="pool",
                                                name="pool_ps")

                    # w[b,:] = keys[sigma[b,c]] via one-hot bf16 matmul.
                    # high_priority: the scheduler must slot gathers ahead of
                    # pool matmuls the moment a w bank frees, else each dot
                    # waits on a gather queued behind a ~550ns pool (the PE
                    # and DVE then alternate idling and the PE never ramps).
                    w_ps = psw.tile([BLK, D], f32, tag="w")
                    with tc.high_priority():
                        nc.tensor.matmul(w_ps[:], oh_ts[m][:, c, :],
                                         keys_t[:], start=True, stop=True)

                    # logit column: fused mult + free-axis accumulate on DVE.
                    # The product is written back over the w PSUM bank itself
                    # (dead value) so the DVE adds no SBUF write traffic.
                    nc.vector.scalar_tensor_tensor(
                        w_ps[:], h_ts[m][:, c, :], 1.0, w_ps[:],
                        op0=ALU.mult, op1=ALU.mult,
                        accum_out=logit_ts[m][:, c:c + 1],
                    )

                    # e = exp(logit/KEY_SCALE + pen) (pen as bias)
                    pc = m * K + c
                    nc.scalar.activation(
                        e_ts[m][:, c:c + 1], logit_ts[m][:, c:c + 1],
                        AF.Exp, bias=pen_t[:, pc:pc + 1], scale=1.0)

                    # diag(e_c) for the pooling matmul
                    diag_t = diag_pool.tile([BLK, BLK], bf16, tag="diag")
                    nc.gpsimd.affine_select(
                        out=diag_t[:],
                        in_=e_ts[m][:, c:c + 1].broadcast_to([BLK, BLK]),
                        compare_op=ALU.is_equal,
                        fill=0.0,
                        base=0,
                        pattern=[[-1, BLK]],
                        channel_multiplier=1,
                    )
                    diag_ts[g] = diag_t

              for g in range(base, base + GRP):
                j = g - LAG
                if 0 <= j < total:
                    mj, cj = divmod(j, K)
                    nc.tensor.matmul(pool_ps_s[mj][:], diag_ts[j][:],
                                     h_ts[mj][:, cj, :],
                                     start=(cj == 0), stop=(cj == K - 1))
                    diag_ts[j] = None
                    if cj == K - 1:
                        esum_t = red_pool.tile([BLK, 1], f32, tag="esum")
                        nc.vector.tensor_reduce(
                            esum_t[:], e_ts[mj][:, 0:K],
                            axis=AX.X, op=ALU.add)
                        recip_t = red_pool.tile([BLK, 1], f32, tag="recip")
                        nc.vector.reciprocal(recip_t[:], esum_t[:])
                        out_t = out_pool.tile([BLK, D], f32, tag="out")
                        nc.vector.tensor_scalar_mul(out_t[:],
                                                    pool_ps_s[mj][:],
                                                    recip_t[:])
                        nc.scalar.dma_start(
                            out_d[mj * BLK:(mj + 1) * BLK, :], out_t[:])

    nc.compile()
    return nc


def get_program():
    if "v3" not in _prog_cache:
        _prog_cache["v3"] = _build_program()
    return _prog_cache["v3"]


def prep_core_inputs(h_bk: np.ndarray, sigma_bk: np.ndarray,
                     keys_fp8: np.ndarray):
    """Host-side prep of one core's input map (index/format transforms only).
    h_bk: (B_CORE, K, D) float32, sigma_bk: (B_CORE, K) int."""
    sig = sigma_bk.astype(np.int64).reshape(N_BLOCKS, BLK, K)

    # one-hot oh[m, a, c, p] = (sigma[m, p, c] == a); sigma == N_AGENTS
    # (unassigned) matches nothing
    sig_mcp = sig.transpose(0, 2, 1)                       # (m, c, p)
    oh = (sig_mcp[:, None, :, :] ==
          np.arange(N_AGENTS, dtype=np.int64)[None, :, None, None])
    oh = oh.astype(ml_dtypes.bfloat16)                     # (m, a, c, p)
    oh = np.ascontiguousarray(oh).reshape(N_BLOCKS, N_AGENTS, K * BLK)

    # pen[p, m*K + c]
    pen = np.where(sig < N_AGENTS, np.float32(0.0), np.float32(NEG_BIG))
    pen = np.ascontiguousarray(pen.transpose(1, 0, 2))     # (p, m, c)
    pen = pen.reshape(BLK, N_BLOCKS * K).astype(np.float32)

    h_shuf = np.ascontiguousarray(
        h_bk.reshape(N_BLOCKS, BLK, K * D)).astype(ml_dtypes.bfloat16)

    return {"h": h_shuf, "oh": oh, "pen": pen, "keys": keys_fp8}


def kernel(h, keys, sigma):
    from concourse.bass_utils import run_bass_kernel_spmd

    h = np.asarray(h, dtype=np.float32)
    keys = np.asarray(keys, dtype=np.float32)
    sigma = np.asarray(sigma)

    keys_fp8 = keys.astype(ml_dtypes.bfloat16)

    in_maps = []
    for i in range(N_CORES):
        lo, hi = i * B_CORE, (i + 1) * B_CORE
        in_maps.append(prep_core_inputs(h[lo:hi], sigma[lo:hi], keys_fp8))

    nc = get_program()
    res = run_bass_kernel_spmd(nc, in_maps, list(range(N_CORES)))
    global LAST_EXEC_NS, LAST_RESULT
    LAST_EXEC_NS = res.exec_time_ns
    LAST_RESULT = res
    out = np.concatenate([res.results[i]["out"] for i in range(N_CORES)],
                         axis=0)
    return out.astype(np.float32)


if __name__ == "__main__":
    rng = np.random.default_rng(0)
    h = rng.standard_normal((B, K, D), dtype=np.float32)
    keys = (rng.standard_normal((N_AGENTS, D), dtype=np.float32) * 0.01)
    sigma = rng.integers(0, N_AGENTS + 1, size=(B, K)).astype(np.int32)
    out = kernel(h=h, keys=keys, sigma=sigma)
    print("out", out.shape, out.dtype, float(np.abs(out).mean()))


# revision 33
# speedup vs baseline: 1.0180x; 1.0180x over previous
"""Trainium2 Bass kernel for BlockDiagonalAggregator (moe_routing).

Computes, for each batch row b:
    logit[b,k] = dot(keys[sigma[b,k]], h[b,k,:])   (masked -inf where sigma==64)
    alpha      = softmax_k(logit)
    out[b,:]   = sum_k alpha[b,k] * h[b,k,:]

Distribution: data-parallel over B across 8 NeuronCores (512 rows each),
keys replicated, no collectives (per the data-parallel sharding hint).

Per-core algorithm (v3, diag-E layout):
  - b-block = 128 batch rows; 4 blocks per core; chunk = one slot k of a
    block's 128 rows -> 64 chunks per block, 256 per core.  h is shipped
    bf16 in (block, b-part, k, d) order -- a pure reshape+cast on host --
    so each partition's block-load is one contiguous 64KB run.
  - w gather via one-hot matmul on PE: w = oh_c.T @ keys (bf16), where
    oh[a, b] = (sigma[b, c] == a), built host-side.  sigma==64 (unassigned)
    matches no agent -> w = 0.
  - logit via one fused DVE scalar_tensor_tensor: (h*1)*w with free-axis
    accumulate -> logit column [128,1] fp32.
  - e = exp(logit + pen) on ACT, one [128,1] instr per chunk with the
    pen column as the per-partition bias (pen = -1e9 for unassigned ->
    e = 0, matching the reference's mask+softmax; no max-subtraction
    needed since keys std 0.01 keeps |logit| < ~2).  e stored bf16.
  - GpSimd affine_select expands the e column into diag(e) [128,128] bf16
    (iota p-f == 0 keeps the broadcast e, else fills 0).
  - PE pooling: pool_ps += diag(e_c).T @ h_c accumulated over the block's
    64 chunks in a single PSUM bank -> [128, 512].
  - esum = free-axis reduce of the block's e tile [128,64] (one DVE
    instr; replaces per-chunk esum matmuls), recip on DVE,
    out = pool * recip, DMA out fp32.
  - Software pipeline: pool matmul for chunk g runs LAG chunks behind the
    gather/dot/exp/diag front so the PE never waits on the 4-engine chain.
"""

import numpy as np
import ml_dtypes

# Problem constants (hardcoded: kernel.py must be self-contained)
B, K, D = 4096, 64, 512
N_AGENTS = 64
N_CORES = 8
B_CORE = B // N_CORES            # 512 batch rows per core
BLK = 128                        # batch rows per block (= PE partition dim)
N_BLOCKS = B_CORE // BLK         # 4
NEG_BIG = -1e9
LAG = 8                          # pool-matmul pipeline lag (chunks)
KEY_SCALE = 64.0                 # host pre-scale so fp8 keys stay normal-range

_prog_cache = {}


def _build_program():
    import concourse.bacc as bacc
    import concourse.tile as tile
    import concourse.mybir as mybir

    f32 = mybir.dt.float32
    bf16 = mybir.dt.bfloat16
    fp8 = mybir.dt.float8e4
    AF = mybir.ActivationFunctionType
    ALU = mybir.AluOpType
    AX = mybir.AxisListType
    PM = mybir.MatmulPerfMode

    nc = bacc.Bacc("TRN2", target_bir_lowering=False, debug=False,
                   num_devices=N_CORES)

    h_d = nc.dram_tensor("h", [N_BLOCKS, BLK, K * D], bf16,
                         kind="ExternalInput").ap()
    oh_d = nc.dram_tensor("oh", [N_BLOCKS, N_AGENTS, K * BLK], bf16,
                          kind="ExternalInput").ap()
    pen_d = nc.dram_tensor("pen", [BLK, N_BLOCKS * K], f32,
                           kind="ExternalInput").ap()
    keys_d = nc.dram_tensor("keys", [N_AGENTS, D], bf16,
                            kind="ExternalInput").ap()
    out_d = nc.dram_tensor("out", [B_CORE, D], f32, kind="ExternalOutput").ap()

    hd4 = h_d.rearrange("m p (c d) -> m p c d", d=D)
    ohd4 = oh_d.rearrange("m a (c p) -> m a c p", p=BLK)

    with tile.TileContext(nc) as tc:
        with (
            tc.tile_pool(name="const", bufs=1) as const_pool,
            tc.tile_pool(name="hp", bufs=2) as h_pool,
            tc.tile_pool(name="ohp", bufs=2) as oh_pool,
            tc.tile_pool(name="logitp", bufs=12) as logit_pool,
            tc.tile_pool(name="ep", bufs=12) as e_pool,
            tc.tile_pool(name="esump", bufs=2) as esum_pool,
            tc.tile_pool(name="diagp", bufs=12) as diag_pool,
            tc.tile_pool(name="outp", bufs=2) as out_pool,
            tc.tile_pool(name="redp", bufs=2) as red_pool,
            tc.tile_pool(name="psw", bufs=6, space="PSUM") as psw,
            tc.tile_pool(name="psp", bufs=2, space="PSUM") as psp,
        ):
            keys_t = const_pool.tile([N_AGENTS, D], bf16)
            nc.sync.dma_start(keys_t[:], keys_d[:])
            pen_t = const_pool.tile([BLK, N_BLOCKS * K], f32)
            nc.sync.dma_start(pen_t[:], pen_d[:])

            # per-block state, indexed by block id
            h_ts = [None] * N_BLOCKS
            oh_ts = [None] * N_BLOCKS
            esum_ts = [None] * N_BLOCKS
            pool_ps_s = [None] * N_BLOCKS
            diag_ts = [None] * (N_BLOCKS * K)

            total = N_BLOCKS * K
            GRP = 4   # batch gathers/pools to amortize PE DR<->normal switches
            for base in range(0, total + LAG, GRP):
              for g in range(base, base + GRP):
                if g < total:
                    m, c = divmod(g, K)
                    if c == 0:
                        h_t = h_pool.tile([BLK, K, D], bf16, tag="h")
                        q = K // 4
                        for i in range(4):
                            nc.sync.dma_start(h_t[:, i * q:(i + 1) * q, :],
                                              hd4[m][:, i * q:(i + 1) * q, :])
                        h_ts[m] = h_t
                        oh_t = oh_pool.tile([N_AGENTS, K, BLK], bf16,
                                            tag="oh")
                        nc.sync.dma_start(oh_t[:], ohd4[m])
                        oh_ts[m] = oh_t
                        esum_ts[m] = esum_pool.tile([BLK, 1], f32,
                                                    tag="esum", name="esum_t")
                        pool_ps_s[m] = psp.tile([BLK, D], f32, tag="pool",
                                                name="pool_ps")

                    # w[b,:] = keys[sigma[b,c]] via one-hot bf16 matmul.
                    # high_priority: the scheduler must slot gathers ahead of
                    # pool matmuls the moment a w bank frees, else each dot
                    # waits on a gather queued behind a ~550ns pool (the PE
                    # and DVE then alternate idling and the PE never ramps).
                    w_ps = psw.tile([BLK, D], f32, tag="w")
                    with tc.high_priority():
                        nc.tensor.matmul(w_ps[:], oh_ts[m][:, c, :],
                                         keys_t[:], start=True, stop=True)

                    # logit column: fused mult + free-axis accumulate on DVE.
                    # The product is written back over the w PSUM bank itself
                    # (dead value) so the DVE adds no SBUF write traffic.
                    # Per-chunk private tiles everywhere below: a shared
                    # block tile written column-wise serializes against its
                    # broadcast readers (whole-tile WAR) and puts the
                    # exp->diag chain inside the steady-state critical loop.
                    logit_t = logit_pool.tile([BLK, 1], f32, tag="logit")
                    nc.vector.scalar_tensor_tensor(
                        w_ps[:], h_ts[m][:, c, :], 1.0, w_ps[:],
                        op0=ALU.mult, op1=ALU.mult,
                        accum_out=logit_t[:],
                    )

                    # e = exp(logit + pen) (pen as bias)
                    pc = m * K + c
                    e_t = e_pool.tile([BLK, 1], bf16, tag="e")
                    nc.scalar.activation(
                        e_t[:], logit_t[:],
                        AF.Exp, bias=pen_t[:, pc:pc + 1], scale=1.0)

                    # running esum on GpSimd (in-place add, serial per engine)
                    if c == 0:
                        nc.gpsimd.tensor_copy(esum_ts[m][:], e_t[:])
                    else:
                        nc.gpsimd.tensor_tensor(esum_ts[m][:], esum_ts[m][:],
                                                e_t[:], op=ALU.add)

                    # diag(e_c) for the pooling matmul
                    diag_t = diag_pool.tile([BLK, BLK], bf16, tag="diag")
                    nc.gpsimd.affine_select(
                        out=diag_t[:],
                        in_=e_t[:].broadcast_to([BLK, BLK]),
                        compare_op=ALU.is_equal,
                        fill=0.0,
                        base=0,
                        pattern=[[-1, BLK]],
                        channel_multiplier=1,
                    )
                    diag_ts[g] = diag_t

              for g in range(base, base + GRP):
                j = g - LAG
                if 0 <= j < total:
                    mj, cj = divmod(j, K)
                    nc.tensor.matmul(pool_ps_s[mj][:], diag_ts[j][:],
                                     h_ts[mj][:, cj, :],
                                     start=(cj == 0), stop=(cj == K - 1))
                    diag_ts[j] = None
                    if cj == K - 1:
                        recip_t = red_pool.tile([BLK, 1], f32, tag="recip")
                        nc.vector.reciprocal(recip_t[:], esum_ts[mj][:])
                        out_t = out_pool.tile([BLK, D], f32, tag="out")
                        nc.vector.tensor_scalar_mul(out_t[:],
                                                    pool_ps_s[mj][:],
                                                    recip_t[:])
                        nc.scalar.dma_start(
                            out_d[mj * BLK:(mj + 1) * BLK, :], out_t[:])

    nc.compile()
    return nc


def get_program():
    if "v3" not in _prog_cache:
        _prog_cache["v3"] = _build_program()
    return _prog_cache["v3"]


def prep_core_inputs(h_bk: np.ndarray, sigma_bk: np.ndarray,
                     keys_fp8: np.ndarray):
    """Host-side prep of one core's input map (index/format transforms only).
    h_bk: (B_CORE, K, D) float32, sigma_bk: (B_CORE, K) int."""
    sig = sigma_bk.astype(np.int64).reshape(N_BLOCKS, BLK, K)

    # one-hot oh[m, a, c, p] = (sigma[m, p, c] == a); sigma == N_AGENTS
    # (unassigned) matches nothing
    sig_mcp = sig.transpose(0, 2, 1)                       # (m, c, p)
    oh = (sig_mcp[:, None, :, :] ==
          np.arange(N_AGENTS, dtype=np.int64)[None, :, None, None])
    oh = oh.astype(ml_dtypes.bfloat16)                     # (m, a, c, p)
    oh = np.ascontiguousarray(oh).reshape(N_BLOCKS, N_AGENTS, K * BLK)

    # pen[p, m*K + c]
    pen = np.where(sig < N_AGENTS, np.float32(0.0), np.float32(NEG_BIG))
    pen = np.ascontiguousarray(pen.transpose(1, 0, 2))     # (p, m, c)
    pen = pen.reshape(BLK, N_BLOCKS * K).astype(np.float32)

    h_shuf = np.ascontiguousarray(
        h_bk.reshape(N_BLOCKS, BLK, K * D)).astype(ml_dtypes.bfloat16)

    return {"h": h_shuf, "oh": oh, "pen": pen, "keys": keys_fp8}


def kernel(h, keys, sigma):
    from concourse.bass_utils import run_bass_kernel_spmd

    h = np.asarray(h, dtype=np.float32)
    keys = np.asarray(keys, dtype=np.float32)
    sigma = np.asarray(sigma)

    keys_fp8 = keys.astype(ml_dtypes.bfloat16)

    in_maps = []
    for i in range(N_CORES):
        lo, hi = i * B_CORE, (i + 1) * B_CORE
        in_maps.append(prep_core_inputs(h[lo:hi], sigma[lo:hi], keys_fp8))

    nc = get_program()
    res = run_bass_kernel_spmd(nc, in_maps, list(range(N_CORES)))
    global LAST_EXEC_NS, LAST_RESULT
    LAST_EXEC_NS = res.exec_time_ns
    LAST_RESULT = res
    out = np.concatenate([res.results[i]["out"] for i in range(N_CORES)],
                         axis=0)
    return out.astype(np.float32)


if __name__ == "__main__":
    rng = np.random.default_rng(0)
    h = rng.standard_normal((B, K, D), dtype=np.float32)
    keys = (rng.standard_normal((N_AGENTS, D), dtype=np.float32) * 0.01)
    sigma = rng.integers(0, N_AGENTS + 1, size=(B, K)).astype(np.int32)
    out = kernel(h=h, keys=keys, sigma=sigma)
    print("out", out.shape, out.dtype, float(np.abs(out).mean()))
